# revision 20
# baseline (speedup 1.0000x reference)
import numpy as np

B = 8
SEQ = 4096
D = 1024
N_BASE = 10000.0
N_CORES = 8
SPC = SEQ // N_CORES  # 512 seq rows per core
JT = SPC // 128       # 4 rows per partition per batch
M = B * JT            # 32 free-dim row-blocks per partition
NROW = 128 * M        # 4096 packed rows per core

# (ch_in, out_chunks, x_bufs, y_bufs, out_queue, add_engines, partition_id)
# NOTE: every DMA must cover all 128 partitions in ONE instruction —
# partition-sliced DMA pairs halve the per-SDMA-engine rate (measured).
CFG = (8, (8, 8, 8, 8), 3, 4, "sync", ("vector",), False)

_CACHE = {}


def _compute_pe() -> np.ndarray:
    """Mirror of the reference _pos_encoding (default jax backend, f32)."""
    import jax
    import jax.numpy as jnp

    pos = jnp.arange(SEQ, dtype=jnp.float32)[:, None]
    i = jnp.arange(D // 2, dtype=jnp.float32)
    denom = jnp.power(jnp.float32(N_BASE), 2.0 * i / jnp.float32(D))
    ang = pos / denom
    pe = jnp.stack([jnp.sin(ang), jnp.cos(ang)], axis=-1).reshape(SEQ, D)
    return np.asarray(jax.device_get(pe), dtype=np.float32)


def _repack(x, c):
    # core c's slice -> [128*M, D] with row = (p*B + b)*JT + j, so any
    # column-range of the [128, M, D] view is per-partition contiguous.
    xs = x[:, c * SPC : (c + 1) * SPC, :]
    return np.ascontiguousarray(
        xs.reshape(B, 128, JT, D).transpose(1, 0, 2, 3)
    ).reshape(NROW, D)


def _unpack(yr):
    return (
        yr.reshape(128, B, JT, D)
        .transpose(1, 0, 2, 3)
        .reshape(B, SPC, D)
    )


def _build_program(
    ch_in, out_chunks, x_bufs, y_bufs, out_q, add_engines, partition_id=True
):
    import concourse.bacc as bacc
    import concourse.mybir as mybir
    import concourse.tile as tile

    if isinstance(out_chunks, int):
        out_chunks = (out_chunks,) * (M // out_chunks)
    assert M % ch_in == 0 and sum(out_chunks) == M
    n_in = M // ch_in
    n_out = len(out_chunks)

    def dma(eng, out, in_):
        eng.dma_start(out=out, in_=in_)
    nc = bacc.Bacc("TRN2", enable_partition_id=partition_id)
    f32 = mybir.dt.float32
    bf16 = mybir.dt.bfloat16
    x_in = nc.declare_dram_parameter("x", [NROW, D], f32, isOutput=False)
    pe_in = nc.declare_dram_parameter("pe", [128 * JT, D], f32, isOutput=False)
    y_out = nc.declare_dram_parameter("y", [NROW, D], bf16, isOutput=True)

    xap = x_in.rearrange("(p m) d -> p m d", m=M)
    yap = y_out.rearrange("(p m) d -> p m d", m=M)
    peap = pe_in.rearrange("(p u) d -> p u d", u=JT)

    with tile.TileContext(nc) as tc:
        with (
            tc.tile_pool(name="pe_pool", bufs=1) as pe_pool,
            tc.tile_pool(name="x_pool", bufs=x_bufs or n_in) as x_pool,
            tc.tile_pool(name="y_pool", bufs=min(y_bufs, n_out)) as y_pool,
        ):
            pe_t = pe_pool.tile([128, JT, D], f32)
            dma(nc.sync, pe_t[:], peap)
            xts = []
            for ci in range(n_in):
                xt = x_pool.tile([128, ch_in, D], f32)
                dma(nc.sync, xt[:], xap[:, ci * ch_in : (ci + 1) * ch_in, :])
                xts.append(xt)
            outq = getattr(nc, out_q)
            engs = [getattr(nc, e) for e in add_engines]
            k = 0
            col0 = 0
            for oc, ch_out in enumerate(out_chunks):
                yt = y_pool.tile([128, ch_out, D], bf16)
                for t in range(ch_out // JT):
                    col = col0 + t * JT
                    ci, lc = divmod(col, ch_in)
                    engs[k % len(engs)].tensor_add(
                        yt[:, t * JT : (t + 1) * JT, :],
                        xts[ci][:, lc : lc + JT, :],
                        pe_t[:],
                    )
                    k += 1
                dma(outq, yap[:, col0 : col0 + ch_out, :], yt[:])
                col0 += ch_out
    if not nc.is_finalized():
        nc.finalize()
    return nc


def _get_state(cfg=CFG):
    if cfg not in _CACHE:
        _CACHE[cfg] = _build_program(*cfg)
    if "pe" not in _CACHE:
        _CACHE["pe"] = _compute_pe()
    return _CACHE[cfg], _CACHE["pe"]


def kernel(x, seq_len=None, **_):
    from concourse.bass_utils import run_bass_kernel_spmd

    x = np.asarray(x, dtype=np.float32)
    assert x.shape == (B, SEQ, D)
    if seq_len is not None:
        assert int(np.asarray(seq_len)) == SEQ

    nc, pe = _get_state()
    in_maps = []
    for c in range(N_CORES):
        pes = np.ascontiguousarray(pe[c * SPC : (c + 1) * SPC, :])
        in_maps.append({"x": _repack(x, c), "pe": pes})

    res = run_bass_kernel_spmd(nc, in_maps, list(range(N_CORES))).results

    out = np.empty((B, SEQ, D), dtype=np.float32)
    for c in range(N_CORES):
        yr = np.asarray(res[c]["y"]).astype(np.float32)
        out[:, c * SPC : (c + 1) * SPC, :] = _unpack(yr)
    return out


# revision 21
# speedup vs baseline: 1.1271x; 1.1271x over previous
import numpy as np

B = 8
SEQ = 4096
D = 1024
N_BASE = 10000.0
N_CORES = 8
SPC = SEQ // N_CORES  # 512 seq rows per core
JT = SPC // 128       # 4 rows per partition per batch
M = B * JT            # 32 free-dim row-blocks per partition
NROW = 128 * M        # 4096 packed rows per core

# (ch_in, out_chunks, x_bufs, y_bufs, out_queue, add_engines, partition_id)
# NOTE: every DMA must cover all 128 partitions in ONE instruction —
# partition-sliced DMA pairs halve the per-SDMA-engine rate (measured).
CFG = (8, (8, 8, 8, 8), 3, 4, "sync", ("vector",), False)

_CACHE = {}


def _compute_pe() -> np.ndarray:
    """Mirror of the reference _pos_encoding (default jax backend, f32)."""
    try:
        import jax
        import jax.numpy as jnp

        pos = jnp.arange(SEQ, dtype=jnp.float32)[:, None]
        i = jnp.arange(D // 2, dtype=jnp.float32)
        denom = jnp.power(jnp.float32(N_BASE), 2.0 * i / jnp.float32(D))
        ang = pos / denom
        pe = jnp.stack([jnp.sin(ang), jnp.cos(ang)], axis=-1).reshape(SEQ, D)
        return np.asarray(jax.device_get(pe), dtype=np.float32)
    except Exception:
        pos = np.arange(SEQ, dtype=np.float32)[:, None]
        i = np.arange(D // 2, dtype=np.float32)
        denom = np.power(np.float32(N_BASE), 2.0 * i / np.float32(D))
        ang = (pos / denom).astype(np.float32)
        pe = np.stack(
            [np.sin(ang), np.cos(ang)], axis=-1
        ).reshape(SEQ, D)
        return np.ascontiguousarray(pe, dtype=np.float32)


def _repack(x, c):
    # core c's slice -> [128*M, D] with row = (p*B + b)*JT + j, so any
    # column-range of the [128, M, D] view is per-partition contiguous.
    xs = x[:, c * SPC : (c + 1) * SPC, :]
    return np.ascontiguousarray(
        xs.reshape(B, 128, JT, D).transpose(1, 0, 2, 3)
    ).reshape(NROW, D)


def _unpack(yr):
    return (
        yr.reshape(128, B, JT, D)
        .transpose(1, 0, 2, 3)
        .reshape(B, SPC, D)
    )


def _build_program(
    ch_in, out_chunks, x_bufs, y_bufs, out_q, add_engines, partition_id=True
):
    import concourse.bacc as bacc
    import concourse.mybir as mybir
    import concourse.tile as tile

    if isinstance(out_chunks, int):
        out_chunks = (out_chunks,) * (M // out_chunks)
    assert M % ch_in == 0 and sum(out_chunks) == M
    n_in = M // ch_in
    n_out = len(out_chunks)

    def dma(eng, out, in_):
        eng.dma_start(out=out, in_=in_)
    nc = bacc.Bacc("TRN2", enable_partition_id=partition_id)
    f32 = mybir.dt.float32
    bf16 = mybir.dt.bfloat16
    x_in = nc.declare_dram_parameter("x", [NROW, D], f32, isOutput=False)
    pe_in = nc.declare_dram_parameter("pe", [128 * JT, D], f32, isOutput=False)
    y_out = nc.declare_dram_parameter("y", [NROW, D], bf16, isOutput=True)

    xap = x_in.rearrange("(p m) d -> p m d", m=M)
    yap = y_out.rearrange("(p m) d -> p m d", m=M)
    peap = pe_in.rearrange("(p u) d -> p u d", u=JT)

    with tile.TileContext(nc) as tc:
        with (
            tc.tile_pool(name="pe_pool", bufs=1) as pe_pool,
            tc.tile_pool(name="x_pool", bufs=x_bufs or n_in) as x_pool,
            tc.tile_pool(name="y_pool", bufs=min(y_bufs, n_out)) as y_pool,
        ):
            pe_t = pe_pool.tile([128, JT, D], f32)
            dma(nc.sync, pe_t[:], peap)
            xts = []
            for ci in range(n_in):
                xt = x_pool.tile([128, ch_in, D], f32)
                dma(nc.sync, xt[:], xap[:, ci * ch_in : (ci + 1) * ch_in, :])
                xts.append(xt)
            outq = getattr(nc, out_q)
            engs = [getattr(nc, e) for e in add_engines]
            k = 0
            col0 = 0
            for oc, ch_out in enumerate(out_chunks):
                yt = y_pool.tile([128, ch_out, D], bf16)
                for t in range(ch_out // JT):
                    col = col0 + t * JT
                    ci, lc = divmod(col, ch_in)
                    engs[k % len(engs)].tensor_add(
                        yt[:, t * JT : (t + 1) * JT, :],
                        xts[ci][:, lc : lc + JT, :],
                        pe_t[:],
                    )
                    k += 1
                dma(outq, yap[:, col0 : col0 + ch_out, :], yt[:])
                col0 += ch_out
    if not nc.is_finalized():
        nc.finalize()
    return nc


def _get_state(cfg=CFG):
    if cfg not in _CACHE:
        _CACHE[cfg] = _build_program(*cfg)
    if "pe" not in _CACHE:
        _CACHE["pe"] = _compute_pe()
    return _CACHE[cfg], _CACHE["pe"]


def kernel(x, seq_len=None, **_):
    from concourse.bass_utils import run_bass_kernel_spmd

    x = np.asarray(x, dtype=np.float32)
    assert x.shape == (B, SEQ, D)
    if seq_len is not None:
        assert int(np.asarray(seq_len)) == SEQ

    nc, pe = _get_state()
    in_maps = []
    for c in range(N_CORES):
        pes = np.ascontiguousarray(pe[c * SPC : (c + 1) * SPC, :])
        in_maps.append({"x": _repack(x, c), "pe": pes})

    res = run_bass_kernel_spmd(nc, in_maps, list(range(N_CORES))).results

    out = np.empty((B, SEQ, D), dtype=np.float32)
    for c in range(N_CORES):
        yr = np.asarray(res[c]["y"]).astype(np.float32)
        out[:, c * SPC : (c + 1) * SPC, :] = _unpack(yr)
    return out
